# revision 23
# baseline (speedup 1.0000x reference)
"""BlockSparseMLA Trainium2 kernel — bf16 streaming design, ~91us HW
(baseline was 151us; rel err vs f32 reference ~4.4e-3, threshold 2e-2).

Sharding: 8 cores = 2 batches x 4 seq-quarters; each core runs all 16 heads
for its 512 queries over the 256 selected key positions. Host does block
scoring / top-k / gather / causal-mask build and patches fully-masked rows
(their softmax is uniform over all S -> rank-1 fallback).

Performance notes (from NTFF traces):
  * TRN2's PE clock gate (HAM) is binary 1.2/2.4 GHz: it releases only after
    ~3.4us of gapless matmul issue and re-engages after ~2us idle. The whole
    kernel is therefore structured as ONE unbroken matmul stream: warmup
    matmuls cover the initial DMA wait, stages A(latent)/B(k+RoPE)/C(v) are
    interleaved into stage D's (q-proj) ck loop, and a 12-matmul bridge
    covers the DVE drain before attention. Warm issue pitch is ~215ns per
    512-col bf16 matmul with LDWEIGHTS hidden by the queue lookahead.
  * everything matmul is bf16 (1 cyc/row, half the DMA); inputs are host
    pre-packed [128, N] partition-major blocks so DMA lines are 2-16KB.
  * k-RoPE rotation is folded into a second up-projection weight (wkupP);
    q-RoPE uses a perm-matmul lagged one ck behind the projection.
  * causal mask is folded into the scores psum via negI @ invmask (exp of
    -1e30 underflows to 0), so exp writes the masked expT directly.
  * softmax Z rides the PV matmul: v lhsT = [ones | v_h] puts Z in psum rows
    0:63 (partition offset 0 for reciprocal_approx_fast, which mis-reads at
    a 64-partition base) and y in rows 64:127; fully-masked rows divide to
    garbage and are overwritten by the host fallback.
  * engine split respects measured costs: scalar=exp (1.1ns/col) + copies,
    vector=rope muls / zr / ymul (1.4ns/col, the only DVE that reads PSUM),
    gpsimd=bf16 adds only (2-3ns/col, cannot touch PSUM, chokes on
    64-partition ops). PV lags scores by 3 heads to hide the exp latency.
"""

import sys

import numpy as np

sys.path.insert(0, "/opt/trn_rl_repo")

from contextlib import ExitStack

import ml_dtypes

import concourse.bacc as bacc
import concourse.bass as bass
import concourse.mybir as mybir
import concourse.tile as tile

B, S, D = 2, 2048, 1024
H, HD, R = 16, 64, 128
BLOCK, TOPK = 64, 4
ROPE_BASE = 100000.0
SQ = S // 4
KEYS = TOPK * BLOCK  # 256
CK = D // 128  # output c chunks (2 heads each)
DK = D // 128  # input d chunks
F32 = mybir.dt.float32
F32R = mybir.dt.float32r
BF16 = mybir.dt.bfloat16
NPBF = ml_dtypes.bfloat16

N_WARMUP = 13  # PE warmup matmuls (512 cols each) during initial DMA wait


def _bf(a):
    return np.ascontiguousarray(np.asarray(a, np.float32).astype(NPBF))


def _f32(a):
    return np.ascontiguousarray(a, dtype=np.float32)


def _perm():
    """[128, 128] block-diag rotate-half permutation: out[c] = in[c^32
    within each 64-block]. Used as matmul lhsT for the q rotation."""
    P = np.zeros((128, 128), np.float32)
    for pp in range(128):
        blk, e = divmod(pp, 64)
        s = blk * 64 + (e + 32 if e < 32 else e - 32)
        P[s, pp] = 1.0
    return P


def host_prep(x, w_q, w_kv_down, w_kv_up, w_out, w_scorer):
    """Returns (in_maps for 8 cores, qmin[B], fb_rows[B, D])."""
    x = np.asarray(x, dtype=np.float32)
    nb = S // BLOCK

    reps = x.reshape(B, nb, BLOCK, D).mean(axis=2)
    scores = reps @ np.asarray(w_scorer, np.float32)[0]
    top = np.argsort(-scores, axis=1, kind="stable")[:, :TOPK]
    sel_blocks = np.sort(top, axis=1)
    qmin = sel_blocks[:, 0] * BLOCK
    sel_pos = (
        sel_blocks[:, :, None] * BLOCK + np.arange(BLOCK)[None, None, :]
    ).reshape(B, KEYS)

    # RoPE tables (fp32, mirrors reference._rope_tables)
    half = np.arange(0, HD, 2, dtype=np.float32) / np.float32(HD)
    inv_freq = np.float32(1.0) / np.power(np.float32(ROPE_BASE), half)
    freqs = np.arange(S, dtype=np.float32)[:, None] * inv_freq[None, :]
    emb = np.concatenate([freqs, freqs], axis=1)  # [S, HD]
    cos = np.cos(emb).astype(np.float32)
    sin = np.sin(emb).astype(np.float32)
    sgn = np.where(np.arange(HD) < HD // 2, np.float32(-1.0), np.float32(1.0))
    sins = sin * sgn[None, :]  # signed sin for swap-based rotate_half

    # Fallback row for fully-masked queries: uniform attention over all S
    # positions -> mean(v) -> w_out.
    latent_mean = x.mean(axis=1) @ np.asarray(w_kv_down, np.float32).T  # [B, R]
    v_mean = latent_mean @ np.asarray(w_kv_up, np.float32)[D:].T  # [B, D]
    fb_rows = v_mean @ np.asarray(w_out, np.float32).T  # [B, D]

    w_q = np.asarray(w_q, np.float32)
    w_kv_down = np.asarray(w_kv_down, np.float32)
    w_kv_up = np.asarray(w_kv_up, np.float32)
    w_out = np.asarray(w_out, np.float32)

    # --- shared weight blocks, pre-packed [128, ...] partition-major ---
    # wq_ck[p, dk, c'] = w_q[ck*128+c', dk*128+p]
    wq4 = w_q.reshape(CK, 128, DK, 128).transpose(3, 0, 2, 1)  # [p,ck,dk,c']
    wq_cks = [_bf(wq4[:, ck].reshape(128, DK * 128)) for ck in range(CK)]
    # wkvd[p, dk, r] = w_kv_down[r, dk*128+p]
    wkvd = _bf(w_kv_down.reshape(R, DK, 128).transpose(2, 1, 0).reshape(128, -1))
    # wkup[r, ck, c] = w_kv_up[ck*128+c, r] (k half)
    wkupT = w_kv_up[:D].T  # [R, D]
    wkup = _bf(wkupT.reshape(128, CK, 128).reshape(128, -1))
    # wkupP: columns rotate-half permuted within each head's 64-block
    cidx = np.arange(D)
    e = cidx % 64
    pidx = (cidx // 64) * 64 + np.where(e < 32, e + 32, e - 32)
    wkupP = _bf(wkupT[:, pidx].reshape(128, -1))
    # wvup[r, j] = w_kv_up[D + j, r] (v half, head-major j)
    wvup = _bf(w_kv_up[D:].T.reshape(128, -1))
    # wout[p, ck, j] = w_out[j, ck*128+p]
    wout = _bf(w_out.T.reshape(CK, 128, D).transpose(1, 0, 2).reshape(128, -1))
    perm = _f32(_perm())

    negI = _bf(np.eye(128, dtype=np.float32) * -1e30)
    shared = {}
    for ck in range(CK):
        shared[f"wq{ck}"] = wq_cks[ck]
    shared.update(
        wout=wout, perm=perm,
        _wkvd=wkvd, _wkup=wkup, _wkupP=wkupP, _wvup=wvup, _negI=negI,
    )

    in_maps = []
    for c in range(8):
        b, sq = divmod(c, 4)
        s0 = sq * SQ
        m = dict(shared)
        # xT[p, dk, s] = x[b, s0+s, dk*128+p]; split in half for earlier start
        xTfull = x[b, s0 : s0 + SQ].reshape(SQ, DK, 128).transpose(2, 1, 0)
        m["xTa"] = _bf(xTfull[:, : DK // 2].reshape(128, -1))
        m["xTb"] = _bf(xTfull[:, DK // 2 :].reshape(128, -1))
        m["xsel"] = _bf(
            x[b, sel_pos[b]].reshape(KEYS, DK, 128).transpose(2, 1, 0).reshape(128, -1)
        )
        # invmask[p, mk, s] = 1 where key NOT allowed (sel_pos > s0+s)
        imk = (
            sel_pos[b][:, None] > (s0 + np.arange(SQ))[None, :]
        ).reshape(2, 128, SQ).transpose(1, 0, 2)
        m["tabs"] = np.concatenate(
            [
                _f32(np.tile(cos[s0 : s0 + SQ].T, (2, 1))),
                _f32(np.tile(sins[s0 : s0 + SQ].T, (2, 1))),
                _f32(np.tile(cos[sel_pos[b]].T, (2, 1))),
                _f32(np.tile(sins[sel_pos[b]].T, (2, 1))),
            ],
            axis=1,
        )
        m["blob1"] = np.concatenate([m.pop("xsel"), shared["_wkvd"]], axis=1)
        m["blob2"] = np.concatenate(
            [shared["_wkup"], shared["_wkupP"], shared["_wvup"]], axis=1
        )
        m["blob4"] = np.concatenate(
            [_bf(imk.reshape(128, -1)), shared["_negI"]], axis=1
        )
        for k in list(m):
            if k.startswith("_"):
                del m[k]
        in_maps.append(m)
    return in_maps, qmin, fb_rows


def build_nc():
    nc = bacc.Bacc("TRN2", target_bir_lowering=False)

    HDK = DK // 2
    xTa = nc.dram_tensor("xTa", [128, HDK * SQ], BF16, kind="ExternalInput")
    xTb = nc.dram_tensor("xTb", [128, HDK * SQ], BF16, kind="ExternalInput")
    wq_d = [
        nc.dram_tensor(f"wq{ck}", [128, DK * 128], BF16, kind="ExternalInput")
        for ck in range(CK)
    ]
    wout = nc.dram_tensor("wout", [128, CK * D], BF16, kind="ExternalInput")
    perm = nc.dram_tensor("perm", [128, 128], F32R, kind="ExternalInput")
    # tabs = cosq | sinq | cosk | sink (f32)
    tabs = nc.dram_tensor("tabs", [128, 2 * SQ + 2 * KEYS], F32, kind="ExternalInput")
    blob1 = nc.dram_tensor(
        "blob1", [128, DK * KEYS + DK * R], BF16, kind="ExternalInput"
    )
    blob2 = nc.dram_tensor("blob2", [128, 3 * D], BF16, kind="ExternalInput")
    blob4 = nc.dram_tensor("blob4", [128, 2 * SQ + 128], BF16, kind="ExternalInput")
    out = nc.dram_tensor("out", [SQ, D], F32, kind="ExternalOutput")

    EXP = mybir.ActivationFunctionType.Exp

    with tile.TileContext(nc) as tc, ExitStack() as ctx:
        const = ctx.enter_context(tc.tile_pool(name="const", bufs=1))

        # ---- DMA issue order == criticality order (all on sync queue) ----
        wq_sb = [None] * CK

        def dma_wq(ck):
            t = const.tile([128, DK, 128], BF16, tag=f"wq{ck}")
            nc.sync.dma_start(
                t[:], wq_d[ck][:, :].rearrange("p (k c) -> p k c", k=DK)
            )
            wq_sb[ck] = t

        dma_wq(0)
        xTa_sb = const.tile([128, HDK, SQ], BF16, tag="xTa")
        nc.sync.dma_start(xTa_sb[:], xTa[:, :].rearrange("p (k s) -> p k s", k=HDK))
        xTb_sb = const.tile([128, HDK, SQ], BF16, tag="xTb")
        nc.sync.dma_start(xTb_sb[:], xTb[:, :].rearrange("p (k s) -> p k s", k=HDK))
        perm_sb = const.tile([128, 128], F32R, tag="perm")
        nc.sync.dma_start(perm_sb[:], perm[:, :])
        dma_wq(1)
        dma_wq(2)
        dma_wq(3)
        dma_wq(4)
        tabs_sb = const.tile([128, 2 * SQ + 2 * KEYS], F32, tag="tabs")
        nc.sync.dma_start(tabs_sb[:], tabs[:, :])
        blob1_sb = const.tile([128, DK * (KEYS + R)], BF16, tag="blob1")
        nc.sync.dma_start(blob1_sb[:], blob1[:, :])
        blob2_sb = const.tile([128, 3 * D], BF16, tag="blob2")
        nc.sync.dma_start(blob2_sb[:], blob2[:, :])
        dma_wq(5)
        dma_wq(6)
        dma_wq(7)
        blob4_sb = const.tile([128, 2 * SQ + 128], BF16, tag="blob4")
        nc.sync.dma_start(blob4_sb[:], blob4[:, :])
        wout_sb = const.tile([128, CK, D], BF16, tag="wout")
        nc.sync.dma_start(
            wout_sb[:], wout[:, :].rearrange("p (k c) -> p k c", k=CK)
        )

        # slices
        cosq_sb = tabs_sb[:, 0:SQ]
        sinq_sb = tabs_sb[:, SQ : 2 * SQ]
        cosk_sb = tabs_sb[:, 2 * SQ : 2 * SQ + KEYS]
        sink_sb = tabs_sb[:, 2 * SQ + KEYS : 2 * SQ + 2 * KEYS]
        xsel_sb = blob1_sb[:, 0 : DK * KEYS].rearrange("p (k s) -> p k s", k=DK)
        wkvd_sb = blob1_sb[:, DK * KEYS :].rearrange("p (k r) -> p k r", k=DK)
        wkup_sb = blob2_sb[:, 0:D].rearrange("p (k c) -> p k c", k=CK)
        wkupP_sb = blob2_sb[:, D : 2 * D].rearrange("p (k c) -> p k c", k=CK)
        wvup_sb = blob2_sb[:, 2 * D : 3 * D].rearrange("p (k c) -> p k c", k=2)
        imask_sb = blob4_sb[:, 0 : 2 * SQ].rearrange("p (m s) -> p m s", m=2)
        negI_sb = blob4_sb[:, 2 * SQ : 2 * SQ + 128]

        # ---- persistent intermediates ----
        lat_sb = const.tile([128, KEYS], BF16, tag="lat")
        kT_sb = const.tile([128, CK, KEYS], BF16, tag="kT")
        # v_sb[:, mk, h, 0:64] = ones (Z); [.., 64:128] = v head h (chunk mk)
        v_sb = const.tile([128, 2, H, 128], BF16, tag="v")
        qTr_sb = const.tile([128, CK, SQ], BF16, tag="qTr")
        yT_sb = const.tile([128, CK, SQ], BF16, tag="yT")
        warm_lhs = const.tile([128, 128], BF16, tag="wl")
        warm_rhs = const.tile([128, 512], BF16, tag="wr")

        nc.gpsimd.memset(warm_lhs[:], 0.0)
        nc.gpsimd.memset(warm_rhs[:], 0.0)
        nc.gpsimd.memset(v_sb[:], 0.0)
        nc.gpsimd.memset(v_sb[:, :, :, 0:64], 1.0)

        with (
            tc.tile_pool(name="ps_d", bufs=4, space="PSUM") as ps_d,
            tc.tile_pool(name="ps_s", bufs=4, space="PSUM") as ps_s,
            tc.tile_pool(name="qraw_pool", bufs=3) as qraw_pool,
            tc.tile_pool(name="scr", bufs=4) as scr,
        ):
            # ---- PE warmup: release the HAM clock gate while DMAs land
            wps = ps_d.tile([128, SQ], F32, tag="b")
            for i in range(N_WARMUP):
                nc.tensor.matmul(
                    wps[:],
                    warm_lhs[:],
                    warm_rhs[:],
                    start=(i == 0),
                    stop=(i == N_WARMUP - 1),
                )

            # ---- stage D: qT chunks (ck-outer, dk accumulation) + RoPE.
            # rot matmuls lag one ck; A/B/C matmuls fill D's stream so the
            # PE never idles (idle >2us re-engages the HAM clock gate).
            def emit_rot(ck, q_pss):
                qraw = qraw_pool.tile([128, SQ], F32R, tag="qraw")
                nc.scalar.copy(qraw[:], q_pss[:])
                rot_ps = ps_d.tile([128, SQ], F32, tag="b")
                nc.tensor.matmul(
                    rot_ps[:], perm_sb[:], qraw[:], start=True, stop=True
                )
                qt1 = scr.tile([128, SQ], BF16, tag="qt1")
                nc.vector.tensor_mul(qt1[:], qraw[:], cosq_sb)
                qt2 = scr.tile([128, SQ], BF16, tag="qt2")
                nc.vector.tensor_mul(qt2[:], rot_ps[:], sinq_sb)
                nc.gpsimd.tensor_add(qTr_sb[:, ck, :], qt1[:], qt2[:])

            def emit_A():
                lat_full = ps_s.tile([128, SQ], F32, tag="b")
                lat_ps = lat_full[:, :KEYS]
                for dk in range(DK):
                    nc.tensor.matmul(
                        lat_ps,
                        wkvd_sb[:, dk, :],
                        xsel_sb[:, dk, :],
                        start=(dk == 0),
                        stop=(dk == DK - 1),
                    )
                nc.scalar.copy(lat_sb[:], lat_ps)

            def emit_B(cks):
                for ck in cks:
                    kraw_full = ps_s.tile([128, SQ], F32, tag="b")
                    kraw_ps = kraw_full[:, :KEYS]
                    nc.tensor.matmul(
                        kraw_ps, wkup_sb[:, ck, :], lat_sb[:], start=True, stop=True
                    )
                    krot_full = ps_s.tile([128, SQ], F32, tag="b")
                    krot_ps = krot_full[:, :KEYS]
                    nc.tensor.matmul(
                        krot_ps, wkupP_sb[:, ck, :], lat_sb[:], start=True, stop=True
                    )
                    kt1 = scr.tile([128, KEYS], BF16, tag="kt1")
                    nc.vector.tensor_mul(kt1[:], kraw_ps, cosk_sb)
                    kt2 = scr.tile([128, KEYS], BF16, tag="kt2")
                    nc.vector.tensor_mul(kt2[:], krot_ps, sink_sb)
                    nc.gpsimd.tensor_add(kT_sb[:, ck, :], kt1[:], kt2[:])

            def emit_C():
                for mk in range(2):
                    for half in range(2):
                        v_ps = ps_s.tile([128, 512], F32, tag="b")
                        nc.tensor.matmul(
                            v_ps[:],
                            lat_sb[:, mk * 128 : (mk + 1) * 128],
                            wvup_sb[:, half, :],
                            start=True,
                            stop=True,
                        )
                        dst = v_sb[:, mk, half * 8 : (half + 1) * 8, 64:128]
                        vsrc = v_ps[:].rearrange("p (h c) -> p h c", h=8)
                        nc.scalar.copy(dst, vsrc)

            q_pss = []
            for ck in range(CK):
                q_ps = ps_d.tile([128, SQ], F32, tag="b")
                for dk in range(DK):
                    xt = xTa_sb if dk < HDK else xTb_sb
                    nc.tensor.matmul(
                        q_ps[:],
                        wq_sb[ck][:, dk, :],
                        xt[:, dk % HDK, :],
                        start=(dk == 0),
                        stop=(dk == DK - 1),
                    )
                q_pss.append(q_ps)
                if ck >= 1:
                    emit_rot(ck - 1, q_pss[ck - 1])
                if ck == 4:
                    emit_A()
                elif ck == 5:
                    emit_B([0, 1])
                elif ck == 6:
                    emit_C()
                elif ck == 7:
                    emit_rot(CK - 1, q_pss[CK - 1])
                    emit_B([2, 3])
                    emit_B([4, 5])
                    emit_B([6, 7])
            # bridge: keep the PE busy while the rope-combine DVE work drains
            wps2 = ps_s.tile([128, SQ], F32, tag="b")
            for i in range(22):
                nc.tensor.matmul(
                    wps2[:],
                    warm_lhs[:],
                    warm_rhs[:],
                    start=(i == 0),
                    stop=(i == 21),
                )

        # ================= stage E: attention per head ====================
        # Mask folded into the scores psum (negI @ invmask accumulates -1e30
        # into masked lanes) so exp -> expT directly; PV lhsT = [ones | v_h]
        # puts Z in psum rows 0:63 (offset-0 for reciprocal) and y in 64:127.
        with (
            tc.tile_pool(name="ps_sc", bufs=2, space="PSUM") as ps_sc,
            tc.tile_pool(name="ps_pv", bufs=4, space="PSUM") as ps_pv,
            tc.tile_pool(name="epool", bufs=3) as epool,
            tc.tile_pool(name="ework", bufs=3) as ework,
        ):
            exps = {}

            def emit_scores(h):
                p, hi = divmod(h, 2)
                pb = hi * 64
                sc_ps = ps_sc.tile([128, 2, SQ], F32, tag="sc")
                for mk in range(2):
                    nc.tensor.matmul(
                        sc_ps[:, mk, :],
                        negI_sb,
                        imask_sb[:, mk, :],
                        start=True,
                        stop=False,
                    )
                    nc.tensor.matmul(
                        sc_ps[:, mk, :],
                        kT_sb[pb : pb + 64, p, mk * 128 : (mk + 1) * 128],
                        qTr_sb[pb : pb + 64, p, :],
                        start=False,
                        stop=True,
                    )
                expT = epool.tile([128, 2, SQ], BF16, tag="expT")
                nc.scalar.activation(
                    expT[:].rearrange("p m s -> p (m s)"),
                    sc_ps[:].rearrange("p m s -> p (m s)"),
                    EXP,
                    scale=0.125,
                )
                exps[h] = expT

            def emit_pv(h):
                p, hi = divmod(h, 2)
                pb = hi * 64
                expT = exps.pop(h)
                pv_ps = ps_pv.tile([128, SQ], F32, tag="pv")
                for mk in range(2):
                    nc.tensor.matmul(
                        pv_ps[:],
                        v_sb[:, mk, h, :],
                        expT[:, mk, :],
                        start=(mk == 0),
                        stop=(mk == 1),
                    )
                zr = ework.tile([64, SQ], F32, tag="zr")
                nc.vector.reciprocal_approx_fast(zr[:], pv_ps[0:64, :])
                nc.vector.tensor_mul(
                    yT_sb[pb : pb + 64, p, :], pv_ps[64:128, :], zr[:]
                )

            for h in range(H):
                emit_scores(h)
                if h >= 3:
                    emit_pv(h - 3)
            for h in range(H - 3, H):
                emit_pv(h)

        # ================= stage F: out = yT.T @ wout (st-outer) ==========
        # 1024-col bf16 matmuls (max moving size) amortize the per-matmul
        # LDWEIGHTS serialization; output psum spans 2 banks.
        with (
            tc.tile_pool(name="ps_w", bufs=4, space="PSUM") as ps_w,
            tc.tile_pool(name="ost", bufs=4) as ost,
        ):
            for st in range(4):
                o_sb = ost.tile([128, D], F32, tag="osb")
                for dh in range(2):
                    o_ps = ps_w.tile([128, 512], F32, tag="ops")
                    for ck in range(CK):
                        nc.tensor.matmul(
                            o_ps[:],
                            yT_sb[:, ck, st * 128 : (st + 1) * 128],
                            wout_sb[:, ck, dh * 512 : (dh + 1) * 512],
                            start=(ck == 0),
                            stop=(ck == CK - 1),
                        )
                    if dh == 0:
                        nc.scalar.copy(o_sb[:, 0:512], o_ps[:])
                    else:
                        nc.vector.tensor_copy(o_sb[:, 512:1024], o_ps[:])
                nc.sync.dma_start(out[st * 128 : (st + 1) * 128, :], o_sb[:])

    nc.compile()
    return nc


_NC_CACHE = {}


def _get_nc():
    if "nc" not in _NC_CACHE:
        _NC_CACHE["nc"] = build_nc()
    return _NC_CACHE["nc"]


TRACE = False  # set by test harness to capture an NTFF profile
LAST_RESULTS = None


def kernel(x, w_q, w_kv_down, w_kv_up, w_out, w_scorer):
    global LAST_RESULTS
    from concourse.bass_utils import run_bass_kernel_spmd

    in_maps, qmin, fb_rows = host_prep(x, w_q, w_kv_down, w_kv_up, w_out, w_scorer)
    nc = _get_nc()
    res = run_bass_kernel_spmd(nc, in_maps, core_ids=list(range(8)), trace=TRACE)
    LAST_RESULTS = res
    out = np.empty((B, S, D), np.float32)
    for c in range(8):
        b, sq = divmod(c, 4)
        out[b, sq * SQ : (sq + 1) * SQ] = res.results[c]["out"]
    for b in range(B):
        if qmin[b] > 0:
            out[b, : qmin[b]] = fb_rows[b]
    return out


# revision 24
# speedup vs baseline: 1.0817x; 1.0817x over previous
"""BlockSparseMLA Trainium2 kernel — bf16 streaming design, ~91us HW
(baseline was 151us; rel err vs f32 reference ~4.4e-3, threshold 2e-2).

Sharding: 8 cores = 2 batches x 4 seq-quarters; each core runs all 16 heads
for its 512 queries over the 256 selected key positions. Host does block
scoring / top-k / gather / causal-mask build and patches fully-masked rows
(their softmax is uniform over all S -> rank-1 fallback).

Performance notes (from NTFF traces):
  * TRN2's PE clock gate (HAM) is binary 1.2/2.4 GHz: it releases only after
    ~3.4us of gapless matmul issue and re-engages after ~2us idle. The whole
    kernel is therefore structured as ONE unbroken matmul stream: warmup
    matmuls cover the initial DMA wait, stages A(latent)/B(k+RoPE)/C(v) are
    interleaved into stage D's (q-proj) ck loop, and a 12-matmul bridge
    covers the DVE drain before attention. Warm issue pitch is ~215ns per
    512-col bf16 matmul with LDWEIGHTS hidden by the queue lookahead.
  * everything matmul is bf16 (1 cyc/row, half the DMA); inputs are host
    pre-packed [128, N] partition-major blocks so DMA lines are 2-16KB.
  * k-RoPE rotation is folded into a second up-projection weight (wkupP);
    q-RoPE uses a perm-matmul lagged one ck behind the projection.
  * causal mask is folded into the scores psum via negI @ invmask (exp of
    -1e30 underflows to 0), so exp writes the masked expT directly.
  * softmax Z rides the PV matmul: v lhsT = [ones | v_h] puts Z in psum rows
    0:63 (partition offset 0 for reciprocal_approx_fast, which mis-reads at
    a 64-partition base) and y in rows 64:127; fully-masked rows divide to
    garbage and are overwritten by the host fallback.
  * engine split respects measured costs: scalar=exp (1.1ns/col) + copies,
    vector=rope muls / zr / ymul (1.4ns/col, the only DVE that reads PSUM),
    gpsimd=bf16 adds only (2-3ns/col, cannot touch PSUM, chokes on
    64-partition ops). PV lags scores by 3 heads to hide the exp latency.
"""

import sys

import numpy as np

sys.path.insert(0, "/opt/trn_rl_repo")

from contextlib import ExitStack

import ml_dtypes

import concourse.bacc as bacc
import concourse.bass as bass
import concourse.mybir as mybir
import concourse.tile as tile

B, S, D = 2, 2048, 1024
H, HD, R = 16, 64, 128
BLOCK, TOPK = 64, 4
ROPE_BASE = 100000.0
SQ = S // 4
KEYS = TOPK * BLOCK  # 256
CK = D // 128  # output c chunks (2 heads each)
DK = D // 128  # input d chunks
F32 = mybir.dt.float32
F32R = mybir.dt.float32r
BF16 = mybir.dt.bfloat16
NPBF = ml_dtypes.bfloat16

N_WARMUP = 13  # PE warmup matmuls (512 cols each) during initial DMA wait


def _bf(a):
    return np.ascontiguousarray(np.asarray(a, np.float32).astype(NPBF))


def _f32(a):
    return np.ascontiguousarray(a, dtype=np.float32)


def _perm():
    """[128, 128] block-diag rotate-half permutation: out[c] = in[c^32
    within each 64-block]. Used as matmul lhsT for the q rotation."""
    P = np.zeros((128, 128), np.float32)
    for pp in range(128):
        blk, e = divmod(pp, 64)
        s = blk * 64 + (e + 32 if e < 32 else e - 32)
        P[s, pp] = 1.0
    return P


def host_prep(x, w_q, w_kv_down, w_kv_up, w_out, w_scorer):
    """Returns (in_maps for 8 cores, qmin[B], fb_rows[B, D])."""
    x = np.asarray(x, dtype=np.float32)
    nb = S // BLOCK

    reps = x.reshape(B, nb, BLOCK, D).mean(axis=2)
    scores = reps @ np.asarray(w_scorer, np.float32)[0]
    top = np.argsort(-scores, axis=1, kind="stable")[:, :TOPK]
    sel_blocks = np.sort(top, axis=1)
    qmin = sel_blocks[:, 0] * BLOCK
    sel_pos = (
        sel_blocks[:, :, None] * BLOCK + np.arange(BLOCK)[None, None, :]
    ).reshape(B, KEYS)

    # RoPE tables (fp32, mirrors reference._rope_tables)
    half = np.arange(0, HD, 2, dtype=np.float32) / np.float32(HD)
    inv_freq = np.float32(1.0) / np.power(np.float32(ROPE_BASE), half)
    freqs = np.arange(S, dtype=np.float32)[:, None] * inv_freq[None, :]
    emb = np.concatenate([freqs, freqs], axis=1)  # [S, HD]
    cos = np.cos(emb).astype(np.float32)
    sin = np.sin(emb).astype(np.float32)
    sgn = np.where(np.arange(HD) < HD // 2, np.float32(-1.0), np.float32(1.0))
    sins = sin * sgn[None, :]  # signed sin for swap-based rotate_half

    # Fallback row for fully-masked queries: uniform attention over all S
    # positions -> mean(v) -> w_out.
    latent_mean = x.mean(axis=1) @ np.asarray(w_kv_down, np.float32).T  # [B, R]
    v_mean = latent_mean @ np.asarray(w_kv_up, np.float32)[D:].T  # [B, D]
    fb_rows = v_mean @ np.asarray(w_out, np.float32).T  # [B, D]

    w_q = np.asarray(w_q, np.float32)
    w_kv_down = np.asarray(w_kv_down, np.float32)
    w_kv_up = np.asarray(w_kv_up, np.float32)
    w_out = np.asarray(w_out, np.float32)

    # --- shared weight blocks, pre-packed [128, ...] partition-major ---
    # wq_ck[p, dk, c'] = w_q[ck*128+c', dk*128+p]
    wq4 = w_q.reshape(CK, 128, DK, 128).transpose(3, 0, 2, 1)  # [p,ck,dk,c']
    wq_cks = [_bf(wq4[:, ck].reshape(128, DK * 128)) for ck in range(CK)]
    # wkvd[p, dk, r] = w_kv_down[r, dk*128+p]
    wkvd = _bf(w_kv_down.reshape(R, DK, 128).transpose(2, 1, 0).reshape(128, -1))
    # wkup[r, ck, c] = w_kv_up[ck*128+c, r] (k half)
    wkupT = w_kv_up[:D].T  # [R, D]
    wkup = _bf(wkupT.reshape(128, CK, 128).reshape(128, -1))
    # wkupP: columns rotate-half permuted within each head's 64-block
    cidx = np.arange(D)
    e = cidx % 64
    pidx = (cidx // 64) * 64 + np.where(e < 32, e + 32, e - 32)
    wkupP = _bf(wkupT[:, pidx].reshape(128, -1))
    # wvup[r, j] = w_kv_up[D + j, r] (v half, head-major j)
    wvup = _bf(w_kv_up[D:].T.reshape(128, -1))
    # wout[p, ck, j] = w_out[j, ck*128+p]
    wout = _bf(w_out.T.reshape(CK, 128, D).transpose(1, 0, 2).reshape(128, -1))
    perm = _f32(_perm())

    negI = _bf(np.eye(128, dtype=np.float32) * -1e30)
    shared = {}
    for ck in range(CK):
        shared[f"wq{ck}"] = wq_cks[ck]
    shared.update(
        wout=wout, perm=perm,
        _wkvd=wkvd, _wkup=wkup, _wkupP=wkupP, _wvup=wvup, _negI=negI,
    )

    in_maps = []
    for c in range(8):
        b, sq = divmod(c, 4)
        s0 = sq * SQ
        m = dict(shared)
        # xT[p, dk, s] = x[b, s0+s, dk*128+p]; split in half for earlier start
        xTfull = x[b, s0 : s0 + SQ].reshape(SQ, DK, 128).transpose(2, 1, 0)
        m["xTa"] = _bf(xTfull[:, : DK // 2].reshape(128, -1))
        m["xTb"] = _bf(xTfull[:, DK // 2 :].reshape(128, -1))
        m["xsel"] = _bf(
            x[b, sel_pos[b]].reshape(KEYS, DK, 128).transpose(2, 1, 0).reshape(128, -1)
        )
        # invmask[p, mk, s] = 1 where key NOT allowed (sel_pos > s0+s)
        imk = (
            sel_pos[b][:, None] > (s0 + np.arange(SQ))[None, :]
        ).reshape(2, 128, SQ).transpose(1, 0, 2)
        m["tabs"] = np.concatenate(
            [
                _f32(np.tile(cos[s0 : s0 + SQ].T, (2, 1))),
                _f32(np.tile(sins[s0 : s0 + SQ].T, (2, 1))),
                _f32(np.tile(cos[sel_pos[b]].T, (2, 1))),
                _f32(np.tile(sins[sel_pos[b]].T, (2, 1))),
            ],
            axis=1,
        )
        m["blob1"] = np.concatenate([m.pop("xsel"), shared["_wkvd"]], axis=1)
        m["blob2"] = np.concatenate(
            [shared["_wkup"], shared["_wkupP"], shared["_wvup"]], axis=1
        )
        m["blob4"] = np.concatenate(
            [_bf(imk.reshape(128, -1)), shared["_negI"]], axis=1
        )
        for k in list(m):
            if k.startswith("_"):
                del m[k]
        in_maps.append(m)
    return in_maps, qmin, fb_rows


def build_nc():
    nc = bacc.Bacc("TRN2", target_bir_lowering=False)

    HDK = DK // 2
    xTa = nc.dram_tensor("xTa", [128, HDK * SQ], BF16, kind="ExternalInput")
    xTb = nc.dram_tensor("xTb", [128, HDK * SQ], BF16, kind="ExternalInput")
    wq_d = [
        nc.dram_tensor(f"wq{ck}", [128, DK * 128], BF16, kind="ExternalInput")
        for ck in range(CK)
    ]
    wout = nc.dram_tensor("wout", [128, CK * D], BF16, kind="ExternalInput")
    perm = nc.dram_tensor("perm", [128, 128], F32R, kind="ExternalInput")
    # tabs = cosq | sinq | cosk | sink (f32)
    tabs = nc.dram_tensor("tabs", [128, 2 * SQ + 2 * KEYS], F32, kind="ExternalInput")
    blob1 = nc.dram_tensor(
        "blob1", [128, DK * KEYS + DK * R], BF16, kind="ExternalInput"
    )
    blob2 = nc.dram_tensor("blob2", [128, 3 * D], BF16, kind="ExternalInput")
    blob4 = nc.dram_tensor("blob4", [128, 2 * SQ + 128], BF16, kind="ExternalInput")
    out = nc.dram_tensor("out", [SQ, D], F32, kind="ExternalOutput")

    EXP = mybir.ActivationFunctionType.Exp

    with tile.TileContext(nc) as tc, ExitStack() as ctx:
        const = ctx.enter_context(tc.tile_pool(name="const", bufs=1))

        # ---- DMA issue order == criticality order (all on sync queue) ----
        wq_sb = [None] * CK

        def dma_wq(ck):
            t = const.tile([128, DK, 128], BF16, tag=f"wq{ck}")
            nc.sync.dma_start(
                t[:], wq_d[ck][:, :].rearrange("p (k c) -> p k c", k=DK)
            )
            wq_sb[ck] = t

        dma_wq(0)
        xTa_sb = const.tile([128, HDK, SQ], BF16, tag="xTa")
        nc.sync.dma_start(xTa_sb[:], xTa[:, :].rearrange("p (k s) -> p k s", k=HDK))
        xTb_sb = const.tile([128, HDK, SQ], BF16, tag="xTb")
        nc.sync.dma_start(xTb_sb[:], xTb[:, :].rearrange("p (k s) -> p k s", k=HDK))
        perm_sb = const.tile([128, 128], F32R, tag="perm")
        nc.sync.dma_start(perm_sb[:], perm[:, :])
        dma_wq(1)
        dma_wq(2)
        tabs_sb = const.tile([128, 2 * SQ + 2 * KEYS], F32, tag="tabs")
        nc.sync.dma_start(tabs_sb[:], tabs[:, :])
        blob1_sb = const.tile([128, DK * (KEYS + R)], BF16, tag="blob1")
        nc.sync.dma_start(blob1_sb[:], blob1[:, :])
        dma_wq(3)
        dma_wq(4)
        blob2_sb = const.tile([128, 3 * D], BF16, tag="blob2")
        nc.sync.dma_start(blob2_sb[:], blob2[:, :])
        dma_wq(5)
        dma_wq(6)
        dma_wq(7)
        blob4_sb = const.tile([128, 2 * SQ + 128], BF16, tag="blob4")
        nc.sync.dma_start(blob4_sb[:], blob4[:, :])
        wout_sb = const.tile([128, CK, D], BF16, tag="wout")
        nc.sync.dma_start(
            wout_sb[:], wout[:, :].rearrange("p (k c) -> p k c", k=CK)
        )

        # slices
        cosq_sb = tabs_sb[:, 0:SQ]
        sinq_sb = tabs_sb[:, SQ : 2 * SQ]
        cosk_sb = tabs_sb[:, 2 * SQ : 2 * SQ + KEYS]
        sink_sb = tabs_sb[:, 2 * SQ + KEYS : 2 * SQ + 2 * KEYS]
        xsel_sb = blob1_sb[:, 0 : DK * KEYS].rearrange("p (k s) -> p k s", k=DK)
        wkvd_sb = blob1_sb[:, DK * KEYS :].rearrange("p (k r) -> p k r", k=DK)
        wkup_sb = blob2_sb[:, 0:D].rearrange("p (k c) -> p k c", k=CK)
        wkupP_sb = blob2_sb[:, D : 2 * D].rearrange("p (k c) -> p k c", k=CK)
        wvup_sb = blob2_sb[:, 2 * D : 3 * D].rearrange("p (k c) -> p k c", k=2)
        imask_sb = blob4_sb[:, 0 : 2 * SQ].rearrange("p (m s) -> p m s", m=2)
        negI_sb = blob4_sb[:, 2 * SQ : 2 * SQ + 128]

        # ---- persistent intermediates ----
        lat_sb = const.tile([128, KEYS], BF16, tag="lat")
        kT_sb = const.tile([128, CK, KEYS], BF16, tag="kT")
        # v_sb[:, mk, h, 0:64] = ones (Z); [.., 64:128] = v head h (chunk mk)
        v_sb = const.tile([128, 2, H, 128], BF16, tag="v")
        qTr_sb = const.tile([128, CK, SQ], BF16, tag="qTr")
        yT_sb = const.tile([128, CK, SQ], BF16, tag="yT")
        warm_lhs = const.tile([128, 128], BF16, tag="wl")
        warm_rhs = const.tile([128, 512], BF16, tag="wr")

        nc.gpsimd.memset(warm_lhs[:], 0.0)
        nc.gpsimd.memset(warm_rhs[:], 0.0)
        nc.gpsimd.memset(v_sb[:], 0.0)
        nc.gpsimd.memset(v_sb[:, :, :, 0:64], 1.0)

        with (
            tc.tile_pool(name="ps_d", bufs=4, space="PSUM") as ps_d,
            tc.tile_pool(name="ps_s", bufs=4, space="PSUM") as ps_s,
            tc.tile_pool(name="qraw_pool", bufs=3) as qraw_pool,
            tc.tile_pool(name="scr", bufs=4) as scr,
        ):
            # ---- PE warmup: release the HAM clock gate while DMAs land
            wps = ps_d.tile([128, SQ], F32, tag="b")
            for i in range(N_WARMUP):
                nc.tensor.matmul(
                    wps[:],
                    warm_lhs[:],
                    warm_rhs[:],
                    start=(i == 0),
                    stop=(i == N_WARMUP - 1),
                )

            # ---- stage D: qT chunks (ck-outer, dk accumulation) + RoPE.
            # rot matmuls lag one ck; A/B/C matmuls fill D's stream so the
            # PE never idles (idle >2us re-engages the HAM clock gate).
            def emit_rot(ck, q_pss):
                qraw = qraw_pool.tile([128, SQ], F32R, tag="qraw")
                nc.scalar.copy(qraw[:], q_pss[:])
                rot_ps = ps_d.tile([128, SQ], F32, tag="b")
                nc.tensor.matmul(
                    rot_ps[:], perm_sb[:], qraw[:], start=True, stop=True
                )
                qt1 = scr.tile([128, SQ], BF16, tag="qt1")
                nc.vector.tensor_mul(qt1[:], qraw[:], cosq_sb)
                qt2 = scr.tile([128, SQ], BF16, tag="qt2")
                nc.vector.tensor_mul(qt2[:], rot_ps[:], sinq_sb)
                nc.gpsimd.tensor_add(qTr_sb[:, ck, :], qt1[:], qt2[:])

            def emit_A():
                lat_full = ps_s.tile([128, SQ], F32, tag="b")
                lat_ps = lat_full[:, :KEYS]
                for dk in range(DK):
                    nc.tensor.matmul(
                        lat_ps,
                        wkvd_sb[:, dk, :],
                        xsel_sb[:, dk, :],
                        start=(dk == 0),
                        stop=(dk == DK - 1),
                    )
                nc.scalar.copy(lat_sb[:], lat_ps)

            def emit_B(cks):
                for ck in cks:
                    kraw_full = ps_s.tile([128, SQ], F32, tag="b")
                    kraw_ps = kraw_full[:, :KEYS]
                    nc.tensor.matmul(
                        kraw_ps, wkup_sb[:, ck, :], lat_sb[:], start=True, stop=True
                    )
                    krot_full = ps_s.tile([128, SQ], F32, tag="b")
                    krot_ps = krot_full[:, :KEYS]
                    nc.tensor.matmul(
                        krot_ps, wkupP_sb[:, ck, :], lat_sb[:], start=True, stop=True
                    )
                    kt1 = scr.tile([128, KEYS], BF16, tag="kt1")
                    nc.vector.tensor_mul(kt1[:], kraw_ps, cosk_sb)
                    kt2 = scr.tile([128, KEYS], BF16, tag="kt2")
                    nc.vector.tensor_mul(kt2[:], krot_ps, sink_sb)
                    nc.gpsimd.tensor_add(kT_sb[:, ck, :], kt1[:], kt2[:])

            def emit_C():
                for mk in range(2):
                    for half in range(2):
                        v_ps = ps_s.tile([128, 512], F32, tag="b")
                        nc.tensor.matmul(
                            v_ps[:],
                            lat_sb[:, mk * 128 : (mk + 1) * 128],
                            wvup_sb[:, half, :],
                            start=True,
                            stop=True,
                        )
                        dst = v_sb[:, mk, half * 8 : (half + 1) * 8, 64:128]
                        vsrc = v_ps[:].rearrange("p (h c) -> p h c", h=8)
                        if half == 0:
                            nc.scalar.copy(dst, vsrc)
                        else:
                            nc.vector.tensor_copy(dst, vsrc)

            q_pss = []
            for ck in range(CK):
                q_ps = ps_d.tile([128, SQ], F32, tag="b")
                for dk in range(DK):
                    xt = xTa_sb if dk < HDK else xTb_sb
                    nc.tensor.matmul(
                        q_ps[:],
                        wq_sb[ck][:, dk, :],
                        xt[:, dk % HDK, :],
                        start=(dk == 0),
                        stop=(dk == DK - 1),
                    )
                q_pss.append(q_ps)
                if ck >= 1:
                    emit_rot(ck - 1, q_pss[ck - 1])
                if ck == 4:
                    emit_A()
                elif ck == 5:
                    emit_B([0, 1])
                elif ck == 6:
                    emit_C()
                elif ck == 7:
                    emit_rot(CK - 1, q_pss[CK - 1])
                    emit_B([2, 3])
                    emit_B([4, 5])
                    emit_B([6, 7])
            # bridge: keep the PE busy while the rope-combine DVE work drains
            wps2 = ps_s.tile([128, SQ], F32, tag="b")
            for i in range(18):
                nc.tensor.matmul(
                    wps2[:],
                    warm_lhs[:],
                    warm_rhs[:],
                    start=(i == 0),
                    stop=(i == 17),
                )

        # ================= stage E: attention per head ====================
        # Mask folded into the scores psum (negI @ invmask accumulates -1e30
        # into masked lanes) so exp -> expT directly; PV lhsT = [ones | v_h]
        # puts Z in psum rows 0:63 (offset-0 for reciprocal) and y in 64:127.
        with (
            tc.tile_pool(name="ps_sc", bufs=2, space="PSUM") as ps_sc,
            tc.tile_pool(name="ps_pv", bufs=4, space="PSUM") as ps_pv,
            tc.tile_pool(name="epool", bufs=3) as epool,
            tc.tile_pool(name="ework", bufs=3) as ework,
        ):
            exps = {}

            def emit_scores(h):
                p, hi = divmod(h, 2)
                pb = hi * 64
                sc_ps = ps_sc.tile([128, 2, SQ], F32, tag="sc")
                for mk in range(2):
                    nc.tensor.matmul(
                        sc_ps[:, mk, :],
                        negI_sb,
                        imask_sb[:, mk, :],
                        start=True,
                        stop=False,
                    )
                    nc.tensor.matmul(
                        sc_ps[:, mk, :],
                        kT_sb[pb : pb + 64, p, mk * 128 : (mk + 1) * 128],
                        qTr_sb[pb : pb + 64, p, :],
                        start=False,
                        stop=True,
                    )
                expT = epool.tile([128, 2, SQ], BF16, tag="expT")
                nc.scalar.activation(
                    expT[:].rearrange("p m s -> p (m s)"),
                    sc_ps[:].rearrange("p m s -> p (m s)"),
                    EXP,
                    scale=0.125,
                )
                exps[h] = expT

            def emit_pv(h):
                p, hi = divmod(h, 2)
                pb = hi * 64
                expT = exps.pop(h)
                pv_ps = ps_pv.tile([128, SQ], F32, tag="pv")
                for mk in range(2):
                    nc.tensor.matmul(
                        pv_ps[:],
                        v_sb[:, mk, h, :],
                        expT[:, mk, :],
                        start=(mk == 0),
                        stop=(mk == 1),
                    )
                zr = ework.tile([64, SQ], F32, tag="zr")
                nc.vector.reciprocal_approx_fast(zr[:], pv_ps[0:64, :])
                nc.vector.tensor_mul(
                    yT_sb[pb : pb + 64, p, :], pv_ps[64:128, :], zr[:]
                )

            for h in range(H):
                emit_scores(h)
                if h >= 3:
                    emit_pv(h - 3)
            for h in range(H - 3, H):
                emit_pv(h)

        # ================= stage F: out = yT.T @ wout (st-outer) ==========
        # 1024-col bf16 matmuls (max moving size) amortize the per-matmul
        # LDWEIGHTS serialization; output psum spans 2 banks.
        with (
            tc.tile_pool(name="ps_w", bufs=4, space="PSUM") as ps_w,
            tc.tile_pool(name="ost", bufs=4) as ost,
        ):
            for st in range(4):
                o_sb = ost.tile([128, D], F32, tag="osb")
                for dh in range(2):
                    o_ps = ps_w.tile([128, 512], F32, tag="ops")
                    for ck in range(CK):
                        nc.tensor.matmul(
                            o_ps[:],
                            yT_sb[:, ck, st * 128 : (st + 1) * 128],
                            wout_sb[:, ck, dh * 512 : (dh + 1) * 512],
                            start=(ck == 0),
                            stop=(ck == CK - 1),
                        )
                    if dh == 0:
                        nc.scalar.copy(o_sb[:, 0:512], o_ps[:])
                    else:
                        nc.vector.tensor_copy(o_sb[:, 512:1024], o_ps[:])
                nc.sync.dma_start(out[st * 128 : (st + 1) * 128, :], o_sb[:])

    nc.compile()
    return nc


_NC_CACHE = {}


def _get_nc():
    if "nc" not in _NC_CACHE:
        _NC_CACHE["nc"] = build_nc()
    return _NC_CACHE["nc"]


TRACE = False  # set by test harness to capture an NTFF profile
LAST_RESULTS = None


def kernel(x, w_q, w_kv_down, w_kv_up, w_out, w_scorer):
    global LAST_RESULTS
    from concourse.bass_utils import run_bass_kernel_spmd

    in_maps, qmin, fb_rows = host_prep(x, w_q, w_kv_down, w_kv_up, w_out, w_scorer)
    nc = _get_nc()
    res = run_bass_kernel_spmd(nc, in_maps, core_ids=list(range(8)), trace=TRACE)
    LAST_RESULTS = res
    out = np.empty((B, S, D), np.float32)
    for c in range(8):
        b, sq = divmod(c, 4)
        out[b, sq * SQ : (sq + 1) * SQ] = res.results[c]["out"]
    for b in range(B):
        if qmin[b] > 0:
            out[b, : qmin[b]] = fb_rows[b]
    return out
